# revision 3
# baseline (speedup 1.0000x reference)
"""BaiLing attention Trainium2 kernel.

Sharding: 8 cores = 2 (batch) x 4 (tensor-parallel over heads).
Each TP rank r owns q heads 4r..4r+3 and kv head r (GQA group-aligned),
computes its out-projection partial; host sums the 4 partials per batch.

On-chip layout is feature-major [d, s] everywhere:
  - QKV proj:  psum[qkv_col, s] = w_qkv_slice.T @ hidden.T
  - scoresT:   psum[sk, sq]     = k_tile.T @ q_tile    (both [d, *])
  - causal mask: -1e9 accumulated into the scores psum via an
    identity-matmul of a constant mask tile
  - softmax over sk (partition dim): exp on ACT; the denominator is
    accumulated as an elementwise f32 running sum of the exp tiles on
    the otherwise-idle Pool engine, then contracted over partitions
    with ONE full-width ones-matmul pair per (tile, head) into a
    broadcast [128,512] psum (every partition = den).  Small-M one-hot
    matmuls (the old per-exp-tile denominator scheme) run ~1.5x slower
    than full-M matmuls on PE and stall the following weight load, so
    they are avoided entirely.  1/den comes from the DVE
    reciprocal_approx_fast custom op (18 significant bits) and the
    normalize is fused into the PSUM->SBUF copy of the PV output.
  - per-head RMSNorm: sum of squares via the same ones-matmul
    broadcast trick, Ln/Exp on ACT for rsqrt, multiply on DVE.
  - PV:        psum[d, sq]      = vT_tile.T @ probsT
  - out-proj:  psum[s, n]       = oT_slice.T @ w_o_slice; partials are
    stored as fp16 (halves the output DMA traffic; host sums in f64).
Out-proj tiles are emitted one attention tile late so the tensor
engine queue never waits on the normalize chain.  DMA issue is spread
across queues: sync=weights/stores, scalar=hiddens + tail stores, gpsimd=constants/rope tables.
"""

import sys

sys.path.insert(0, "/opt/trn_rl_repo")

import math
from contextlib import ExitStack

import ml_dtypes
import numpy as np

BF = ml_dtypes.bfloat16

import concourse.bass as bass
import concourse.mybir as mybir
import concourse.tile as tile
from concourse import bacc
from concourse.bass_utils import run_bass_kernel_spmd

F32 = mybir.dt.float32
BF16 = mybir.dt.bfloat16
FP16 = mybir.dt.float16
I32 = mybir.dt.int32
AF = mybir.ActivationFunctionType
OP = mybir.AluOpType

H = 2048          # hidden size
S = 2048          # sequence length
D = 128           # head dim
NH_L = 4          # q heads per rank
QW = NH_L * D     # 512 local q width
CW = QW + 2 * D   # 768 local qkv width
P = 128
KO = H // P       # 16 contraction tiles
ST = S // 512     # 4 seq tiles of 512
SM_SCALE = float(D) ** -0.5
EPS = 1e-6
ROPE_THETA = 10000.0
NEG = -1.0e9


def _build():
    nc = bacc.Bacc("TRN2", target_bir_lowering=False, debug=False, num_devices=8)

    hT = nc.dram_tensor("hT", [H, S], BF16, kind="ExternalInput").ap()
    wqkv = nc.dram_tensor("wqkv", [H, CW], BF16, kind="ExternalInput").ap()
    wo = nc.dram_tensor("wo", [QW, H], BF16, kind="ExternalInput").ap()
    cosbt = nc.dram_tensor("cosbt", [P, S], F32, kind="ExternalInput").ap()
    sinbt = nc.dram_tensor("sinbt", [P, S], F32, kind="ExternalInput").ap()
    wqn = nc.dram_tensor("wqn", [D, 1], F32, kind="ExternalInput").ap()
    wkn = nc.dram_tensor("wkn", [D, 1], F32, kind="ExternalInput").ap()
    maskneg = nc.dram_tensor("maskneg", [P, P], BF16, kind="ExternalInput").ap()
    identr = nc.dram_tensor("identr", [P, P], BF16, kind="ExternalInput").ap()
    rmat = nc.dram_tensor("rmat", [P, P], BF16, kind="ExternalInput").ap()
    out = nc.dram_tensor("out", [S, H], FP16, kind="ExternalOutput").ap()

    hT3 = hT.rearrange("(ko p) s -> p ko s", p=P)
    wqkv3 = wqkv.rearrange("(ko p) c -> p ko c", p=P)
    wo3 = wo.rearrange("(ks p) n -> p ks n", p=P)
    out3 = out.rearrange("(st p) n -> p st n", p=P)

    with ExitStack() as ctx:
        tc = ctx.enter_context(tile.TileContext(nc))
        consts = ctx.enter_context(tc.tile_pool(name="consts", bufs=1))
        qkvp = ctx.enter_context(tc.tile_pool(name="qkvp", bufs=1))
        vtp = ctx.enter_context(tc.tile_pool(name="vtp", bufs=1))
        abp = ExitStack()
        csp = abp.enter_context(tc.tile_pool(name="csp", bufs=1))

        # constants on the gpsimd (SWDGE) queue; sync queue starts on
        # weights.  cos/sin rope tables are host-computed (exact) and
        # DMA'd in per-st chunks so the big table transfers don't steal
        # HBM bandwidth from the critical first weight/hidden stream.
        wqn_sb = consts.tile([D, 1], F32)
        nc.gpsimd.dma_start(wqn_sb, wqn)
        wkn_sb = consts.tile([D, 1], F32)
        nc.gpsimd.dma_start(wkn_sb, wkn)
        rmat_sb = consts.tile([P, P], BF16)
        nc.gpsimd.dma_start(rmat_sb, rmat)
        identr_sb = consts.tile([P, P], BF16)
        nc.gpsimd.dma_start(identr_sb, identr)
        maskneg_sb = consts.tile([P, P], BF16)
        nc.gpsimd.dma_start(maskneg_sb, maskneg)
        cosb = csp.tile([P, S], F32)
        sinb = csp.tile([P, S], F32)
        nc.gpsimd.dma_start(cosb[:, 0:512], cosbt[:, 0:512])
        nc.gpsimd.dma_start(sinb[:, 0:512], sinbt[:, 0:512])
        for ts_ in range(1, ST):
            # later tiles' rope tables aren't needed until ~30us in;
            # keep them out of the contended early HBM window
            tsl = slice(512 * ts_, 512 * (ts_ + 1))
            nc.gpsimd.dma_start(cosb[:, tsl], cosbt[:, tsl])
            nc.gpsimd.dma_start(sinb[:, tsl], sinbt[:, tsl])
        ones_sb = consts.tile([P, P], BF16)
        nc.vector.memset(ones_sb, 1.0)
        eps_sb = consts.tile([P, 1], F32)
        nc.vector.memset(eps_sb, EPS)

        q_sb = qkvp.tile([P, NH_L, S], BF16)
        k_sb = qkvp.tile([P, S], BF16)
        vT_sb = vtp.tile([P, KO, P], BF16)

        wqkv_p = abp.enter_context(tc.tile_pool(name="wqkv_p", bufs=1))
        ht_p = abp.enter_context(tc.tile_pool(name="ht_p", bufs=2))
        cpool = abp.enter_context(tc.tile_pool(name="cpool", bufs=1))

        # ---- Phase A: QKV projection + fused norm/rope/v-transpose ----
        # host layout: wqkv cols = [k(128), v(128), q(512)]
        CT_ORDER = [0, 1, 2, 3, 4, 5]  # k, v, then q heads

        def chunk_stage1(x_ch, w_sb, st, ps_c, ps_q, d2):
            """sum-of-squares via full-width ones-matmul into a broadcast
            [128,512] psum (every partition = ssq), rsqrt via Ln/Exp on
            ACT, then the rope rotation.  Emitted one QKV group late so
            the matmuls' DVE inputs are ready when PE reaches them."""
            sl = slice(512 * st, 512 * (st + 1))
            xsq = cpool.tile([P, 512], BF16, tag="ctmp", bufs=4, name="xsq")
            nc.vector.tensor_mul(xsq, x_ch, x_ch)
            sq_ps = ps_q.tile([P, 512], F32, tag="sq", name="sq_ps")
            nc.tensor.matmul(sq_ps, ones_sb, xsq, start=True, stop=True)
            ln_t = cpool.tile([P, 512], F32, tag="lnt", bufs=2, name="ln_t")
            nc.scalar.activation(ln_t, sq_ps, AF.Ln, bias=eps_sb,
                                 scale=1.0 / D)
            rb = cpool.tile([P, 512], BF16, tag="rb", bufs=3, name="rb")
            nc.scalar.activation(rb, ln_t, AF.Exp, scale=-0.5)
            nc.vector.tensor_scalar_mul(x_ch, x_ch, w_sb)
            t1m = cpool.tile([P, 512], BF16, tag="ctmp", bufs=4, name="t1m")
            nc.vector.tensor_mul(t1m, x_ch, cosb[:, sl])
            qr_ps = ps_c.tile([P, 512], F32, tag="qr", name="qr_ps")
            nc.tensor.matmul(qr_ps, rmat_sb, x_ch, start=True, stop=True)
            nc.vector.tensor_tensor(x_ch, qr_ps, sinb[:, sl], OP.mult)
            nc.vector.tensor_tensor(x_ch, x_ch, t1m, OP.add)

            def s2(x_ch=x_ch, rb=rb):
                nc.vector.tensor_tensor(x_ch, x_ch, rb, OP.mult)
            d2.append(s2)

        with nc.named_scope("qkv_proj"):
            with tc.tile_pool(name="ps_a", bufs=4, space="PSUM") as ps_a, \
                 tc.tile_pool(name="ps_c", bufs=2, space="PSUM") as ps_c, \
                 tc.tile_pool(name="ps_q", bufs=2, space="PSUM") as ps_q:
                wq_sb = wqkv_p.tile([P, KO, CW], BF16)
                d1 = []  # stage-1 closures, one group late
                d2 = []  # stage-2 closures, drained one per group
                for st in range(ST):
                    ssl = slice(512 * st, 512 * (st + 1))
                    ht_sb = ht_p.tile([P, KO, 512], BF16, tag="ht",
                                      name=f"ht_{st}")
                    if st == 0:
                        # k/v weight columns and st0 hiddens stream first,
                        # finely interleaved across the sync and vector
                        # queues so the first psum group starts ASAP;
                        # q columns follow.
                        nc.sync.dma_start(
                            wq_sb[:, 0:2, 0 : 2 * D], wqkv3[:, 0:2, 0 : 2 * D])
                        nc.scalar.dma_start(ht_sb[:, 0:2], hT3[:, 0:2, ssl])
                        nc.sync.dma_start(
                            wq_sb[:, 2:4, 0 : 2 * D], wqkv3[:, 2:4, 0 : 2 * D])
                        nc.scalar.dma_start(ht_sb[:, 2:4], hT3[:, 2:4, ssl])
                        nc.sync.dma_start(
                            wq_sb[:, 4:8, 0 : 2 * D], wqkv3[:, 4:8, 0 : 2 * D])
                        nc.scalar.dma_start(ht_sb[:, 4:8], hT3[:, 4:8, ssl])
                        nc.sync.dma_start(
                            wq_sb[:, 8:KO, 0 : 2 * D],
                            wqkv3[:, 8:KO, 0 : 2 * D])
                        nc.scalar.dma_start(ht_sb[:, 8:12], hT3[:, 8:12, ssl])
                        nc.scalar.dma_start(ht_sb[:, 12:16], hT3[:, 12:16, ssl])
                        for qc in range(4):
                            qs = slice(2 * D + P * qc, 2 * D + P * (qc + 1))
                            nc.sync.dma_start(wq_sb[:, :, qs], wqkv3[:, :, qs])
                    else:
                        # later hiddens ride the scalar queue (idle early)
                        nc.scalar.dma_start(ht_sb[:, 0:8], hT3[:, 0:8, ssl])
                        nc.scalar.dma_start(ht_sb[:, 8:16], hT3[:, 8:16, ssl])
                    for ct in CT_ORDER:
                        acc = ps_a.tile([P, 512], F32, tag="qkv_ps",
                                        name=f"qkv_ps_{st}_{ct}")
                        for ko in range(KO):
                            nc.tensor.matmul(
                                acc,
                                wq_sb[:, ko, P * ct : P * (ct + 1)],
                                ht_sb[:, ko],
                                start=(ko == 0),
                                stop=(ko == KO - 1),
                            )
                        # psum->sbuf copies run on ACT (idle here), keeping
                        # DVE for the rope/rms elementwise chain
                        if ct == 0:
                            x_ch, w_sb = k_sb[:, ssl], wkn_sb
                            nc.scalar.copy(x_ch, acc)
                        elif ct == 1:
                            vch = cpool.tile([P, 512], BF16, tag="vch", bufs=2,
                                             name="vch")
                            nc.scalar.copy(vch, acc)
                        else:
                            x_ch, w_sb = q_sb[:, ct - 2, ssl], wqn_sb
                            nc.scalar.copy(x_ch, acc)

                        # run pipelined stages of earlier chunks
                        if d1:
                            d1.pop(0)()
                        if d2:
                            d2.pop(0)()

                        if ct == 1:
                            def vtrans(vch=vch, st=st):
                                for i in range(4):
                                    vt_ps = ps_c.tile([P, P], BF16, tag="qr",
                                                      name="vt_ps")
                                    nc.tensor.transpose(
                                        vt_ps, vch[:, P * i : P * (i + 1)],
                                        identr_sb)
                                    nc.vector.tensor_copy(
                                        vT_sb[:, 4 * st + i], vt_ps)
                            d1.append(vtrans)
                        else:
                            def s1(x_ch=x_ch, w_sb=w_sb, st=st):
                                chunk_stage1(x_ch, w_sb, st, ps_c, ps_q, d2)
                            d1.append(s1)
                while d1:
                    d1.pop(0)()
                while d2:
                    d2.pop(0)()

        abp.close()  # release cos/sin tables + norm temps

        # ------------- Phase E/F: attention + out projection -------------
        with tc.tile_pool(name="otp", bufs=1) as otp, \
             tc.tile_pool(name="wop", bufs=1) as wop, \
             tc.tile_pool(name="expp", bufs=12) as expp, \
             tc.tile_pool(name="esp", bufs=3) as esp, \
             tc.tile_pool(name="ebp", bufs=2) as ebp, \
             tc.tile_pool(name="dip", bufs=2) as dip, \
             tc.tile_pool(name="outp", bufs=3) as outp, \
             tc.tile_pool(name="ps_st", bufs=2, space="PSUM") as ps_st, \
             tc.tile_pool(name="ps_o", bufs=2, space="PSUM") as ps_o, \
             tc.tile_pool(name="ps_dn", bufs=2, space="PSUM") as ps_dn:
            oT_sb = otp.tile([P, NH_L, S], BF16)
            wo_sb = wop.tile([P, NH_L, H], BF16)
            for ks in range(NH_L):
                nc.sync.dma_start(wo_sb[:, ks], wo3[:, ks])

            carry = []  # deferred pv/den/out-proj work from previous heads

            def attn_tile(st, hh):
                n_sk = 4 * st + 4
                qh = q_sb[:, hh, 512 * st : 512 * (st + 1)]
                o_ps = ps_o.tile([P, 512], F32, tag="o_ps", name="o_ps")
                esum = esp.tile([P, 2, 512], F32, tag="esum", name="esum")

                def vis0(j):
                    # first visible sq column of sk-block j within this
                    # 512-wide sq tile; columns below it are fully masked
                    # and never computed/read anywhere
                    return max(0, 128 * (j - 4 * st))

                def emit_pv(eps):
                    for ep, j0 in eps:
                        for u in (0, 1):
                            j = j0 + u
                            c0 = vis0(j)
                            nc.tensor.matmul(o_ps[:, c0:512], vT_sb[:, j],
                                             ep[:, u, c0:512],
                                             start=(j == 0),
                                             stop=(j == n_sk - 1))

                pend = []
                for m in range(2 * st + 2):
                    sT = ps_st.tile([P, 1024], F32, tag="sT", name="sT")
                    for u in (0, 1):
                        j = 2 * m + u
                        half = sT[:, 512 * u : 512 * (u + 1)]
                        if j >= 4 * st:
                            # scores over the visible span (start marks the
                            # whole bank), then the [128,128] staircase mask
                            # accumulated on the diagonal block only
                            c0 = vis0(j)
                            nc.tensor.matmul(
                                half[:, c0:512],
                                k_sb[:, P * j : P * (j + 1)], qh[:, c0:512],
                                start=True, stop=False)
                            nc.tensor.matmul(
                                half[:, c0 : c0 + 128], identr_sb,
                                maskneg_sb,
                                start=False, stop=True)
                        else:
                            nc.tensor.matmul(
                                half, k_sb[:, P * j : P * (j + 1)], qh,
                                start=True, stop=True)
                    ep = expp.tile([P, 2, 512], BF16, tag="ep", name="ep")
                    nc.scalar.activation(ep, sT, AF.Exp, scale=SM_SCALE)
                    # running denominator sum on the Pool engine (f32).
                    # garbage regions of diagonal tiles are skipped; the
                    # one column range never covered at st=0 is zeroed.
                    if m == 0:
                        if st == 0:
                            nc.gpsimd.tensor_copy(esum[:, 0], ep[:, 0])
                            nc.gpsimd.memset(esum[:, 1, 0:128], 0.0)
                            nc.gpsimd.tensor_copy(esum[:, 1, 128:512],
                                                  ep[:, 1, 128:512])
                        else:
                            nc.gpsimd.tensor_copy(esum, ep)
                    elif m < 2 * st:
                        nc.gpsimd.tensor_tensor(esum, esum, ep, OP.add)
                    else:
                        for u in (0, 1):
                            c0 = vis0(2 * m + u)
                            nc.gpsimd.tensor_tensor(
                                esum[:, u, c0:512], esum[:, u, c0:512],
                                ep[:, u, c0:512], OP.add)
                    # interleave the previous head's deferred PV (its exps
                    # are long done) with this head's scores, so PE never
                    # drains a tile's PV right after its exps
                    pend.append((ep, 2 * m))
                    if carry:
                        carry.pop(0)()
                    elif len(pend) > 3:
                        batch, pend = pend[:2], pend[2:]
                        emit_pv(batch)

                def mk(batch):
                    def go():
                        emit_pv(batch)
                    return go

                while pend:
                    batch, pend = pend[:2], pend[2:]
                    carry.append(mk(batch))

                # bf16 shadow of the finished denominator sum for the PE
                # contraction (single rounding; the f32 accumulation order
                # keeps the partial-sum precision)
                esum_bf = ebp.tile([P, 2, 512], BF16, tag="esb",
                                   name="esum_bf")
                nc.gpsimd.tensor_copy(esum_bf, esum)

                osl = oT_sb[:, hh, 512 * st : 512 * (st + 1)]

                def fin_head(esum_bf=esum_bf, o_ps=o_ps, osl=osl):
                    # den broadcast into every psum partition via a
                    # full-width ones matmul (full PE rate), 1/den on DVE,
                    # normalize fused into the PV psum->sbuf copy
                    den_ps = ps_dn.tile([P, 512], F32, tag="dn",
                                        name="den_ps")
                    nc.tensor.matmul(den_ps, ones_sb, esum_bf[:, 0],
                                     start=True, stop=False)
                    nc.tensor.matmul(den_ps, ones_sb, esum_bf[:, 1],
                                     start=False, stop=True)
                    di = dip.tile([P, 512], F32, tag="di", name="di")
                    nc.vector.reciprocal_approx_fast(di, den_ps)
                    nc.vector.tensor_tensor(osl, o_ps, di, OP.mult)
                carry.append(fin_head)
                return osl

            def out_proj(st):
                # the final tile's out-proj is the serial tail: attention is
                # done, so spread its psum groups over the idle scores slots
                # and its copies over both ACT and DVE; non-tail partials
                # are stored as two-tile fp16 batches to halve DMA issues
                tail = st == ST - 1
                with nc.named_scope(f"out_proj_t{st}"):
                    for tp in range(2):
                        if not tail:
                            ob = outp.tile([P, 2, H], FP16, tag="out_sb",
                                           name="out_sb")
                        for ti in range(2):
                            t = 4 * st + 2 * tp + ti
                            if tail:
                                ob1 = outp.tile([P, H], FP16, tag="out_tl",
                                                bufs=2, name="out_tl")
                            for nt in range(4):
                                if tail and nt % 2 == 1:
                                    acc = ps_st.tile([P, 512], F32, tag="sT",
                                                     name="out_ps_b")
                                else:
                                    acc = ps_o.tile([P, 512], F32, tag="o_ps",
                                                    name="out_ps")
                                for ks in range(NH_L):
                                    nc.tensor.matmul(
                                        acc,
                                        oT_sb[:, ks, P * t : P * (t + 1)],
                                        wo_sb[:, ks, 512 * nt : 512 * (nt + 1)],
                                        start=(ks == 0),
                                        stop=(ks == NH_L - 1),
                                    )
                                if tail:
                                    dst = ob1[:, 512 * nt : 512 * (nt + 1)]
                                else:
                                    dst = ob[:, ti, 512 * nt : 512 * (nt + 1)]
                                if tail and nt % 2 == 1:
                                    nc.scalar.copy(dst, acc)
                                else:
                                    nc.vector.tensor_copy(dst, acc)
                                if tail:
                                    # chunked store on alternating queues so
                                    # the last DMAs don't all trail the
                                    # final copy
                                    q = nc.sync if nt % 2 == 0 else nc.scalar
                                    q.dma_start(
                                        out3[:, t, 512 * nt : 512 * (nt + 1)],
                                        dst)
                        if not tail:
                            t0 = 4 * st + 2 * tp
                            nc.sync.dma_start(out3[:, t0 : t0 + 2], ob)

            for st in range(ST):
                for hh in range(NH_L):
                    with nc.named_scope(f"attn_h{hh}_t{st}"):
                        attn_tile(st, hh)

                def fin_st(st=st):
                    out_proj(st)
                carry.append(fin_st)
            while carry:
                carry.pop(0)()

    nc.compile()
    _merge_act_table_loads(nc)
    return nc


def _merge_act_table_loads(nc):
    """Ln(5)/Exp+Copy(0) both live in set 6 (natural_log_exp_and_others);
    bass's per-function table choice alternates 5/0 and reloads tables at
    every Ln<->Exp transition (~1.5us each).  Retarget those loads to
    set 6 and drop the now-redundant reloads."""
    for b in nc.main_func.blocks:
        loaded = None
        keep = []
        for inst in b.instructions:
            if isinstance(inst, mybir.InstLoadActFuncSet):
                tid = inst.act_func_set_id
                if tid in (0, 5):
                    tid = 6
                if tid == loaded:
                    continue
                inst.act_func_set_id = tid
                loaded = tid
            keep.append(inst)
        b.instructions[:] = keep


_NC_CACHE = None


def _get_nc():
    global _NC_CACHE
    if _NC_CACHE is None:
        _NC_CACHE = _build()
    return _NC_CACHE


def _host_inputs(positions, hidden_states, w_qkv, w_o, q_norm_w, k_norm_w):
    """Build the 8 per-core input maps."""
    positions = np.asarray(positions, dtype=np.int32)
    hidden_states = np.asarray(hidden_states, dtype=np.float32)
    w_qkv = np.asarray(w_qkv, dtype=np.float32)
    w_o = np.asarray(w_o, dtype=np.float32)
    q_norm_w = np.asarray(q_norm_w, dtype=np.float32)
    k_norm_w = np.asarray(k_norm_w, dtype=np.float32)

    invf = 1.0 / (ROPE_THETA ** (np.arange(0, D, 2, dtype=np.float64) / D))
    p_idx = np.arange(P).reshape(P, 1)
    c_idx = np.arange(P).reshape(1, P)
    maskneg = np.where(p_idx > c_idx, np.float32(NEG), np.float32(0.0))
    maskneg = maskneg.astype(BF)
    identr = np.eye(P, dtype=BF)
    rmat = np.zeros((P, P), dtype=BF)
    for i in range(64):
        rmat[64 + i, i] = -1.0
        rmat[i, 64 + i] = 1.0
    wqn = q_norm_w.reshape(D, 1)
    wkn = k_norm_w.reshape(D, 1)

    # host-exact rope tables per batch: row r (r%64 = freq) x position
    cosbt, sinbt = [], []
    for g in range(positions.shape[0]):
        ang = np.outer(invf, positions[g].astype(np.float64))  # [64, S]
        c = np.cos(ang).astype(np.float32)
        s = np.sin(ang).astype(np.float32)
        cosbt.append(np.concatenate([c, c], axis=0))
        sinbt.append(np.concatenate([s, s], axis=0))

    in_maps = []
    for core in range(8):
        g, r = core // 4, core % 4
        wq_cols = w_qkv[:, 512 * r : 512 * (r + 1)]
        wk_col = w_qkv[:, 2048 + 128 * r : 2048 + 128 * (r + 1)]
        wv_col = w_qkv[:, 2560 + 128 * r : 2560 + 128 * (r + 1)]
        in_maps.append(
            {
                "hT": np.ascontiguousarray(hidden_states[g].T).astype(BF),
                "wqkv": np.ascontiguousarray(
                    np.concatenate([wk_col, wv_col, wq_cols], axis=1)
                ).astype(BF),
                "wo": np.ascontiguousarray(
                    w_o[512 * r : 512 * (r + 1), :]
                ).astype(BF),
                "cosbt": cosbt[g],
                "sinbt": sinbt[g],
                "wqn": wqn,
                "wkn": wkn,
                "maskneg": maskneg,
                "identr": identr,
                "rmat": rmat,
            }
        )
    return in_maps


def run(trace=False, **inputs):
    nc = _get_nc()
    in_maps = _host_inputs(**inputs)
    res = run_bass_kernel_spmd(nc, in_maps, core_ids=list(range(8)), trace=trace)
    B = inputs["hidden_states"].shape[0]
    out = np.zeros((B, S, H), dtype=np.float64)
    for core in range(8):
        g = core // 4
        out[g] += res.results[core]["out"].astype(np.float64)
    return out.astype(np.float32), res


def kernel(**inputs):
    out, _ = run(trace=False, **inputs)
    return out


# revision 8
# speedup vs baseline: 1.3692x; 1.3692x over previous
"""BaiLing attention Trainium2 kernel.

Sharding: 8 cores = 2 (batch) x 4 (tensor-parallel over heads).
Each TP rank r owns q heads 4r..4r+3 and kv head r (GQA group-aligned),
computes its out-projection partial; host sums the 4 partials per batch.

On-chip layout is feature-major [d, s] everywhere:
  - QKV proj:  psum[qkv_col, s] = w_qkv_slice.T @ hidden.T
  - scoresT:   psum[sk, sq]     = k_tile.T @ q_tile    (both [d, *])
  - causal mask: -1e9 accumulated into the scores psum via an
    identity-matmul of a constant mask tile
  - softmax over sk (partition dim): exp on ACT; the denominator is
    contracted over partitions with full-width [128,128] ones-matmuls
    into a broadcast [128,512] psum per head (every partition = den).
    Small-M one-hot matmuls (the old scheme) run ~1.5x slower on PE
    and stall the following weight load; full-M ones matmuls run at
    peak.  1/den comes from the DVE reciprocal_approx_fast custom op
    (18 significant bits) and the normalize is fused into the
    PSUM->SBUF copy of the PV output.  (Elementwise accumulation of
    the denominator on the GpSimd/Pool engine was tried and is ~4x
    slower than PE here, plus its SBUF traffic slows PE matmuls.)
  - per-head RMSNorm: sum of squares via the same ones-matmul
    broadcast trick, Ln/Exp on ACT for rsqrt, multiply on DVE.
  - PV:        psum[d, sq]      = vT_tile.T @ probsT
  - out-proj:  psum[s, n]       = oT_slice.T @ w_o_slice; partials are
    stored as fp16 (halves the output DMA traffic; host sums in f64).
Out-proj tiles are emitted one attention tile late so the tensor
engine queue never waits on the normalize chain.  DMA issue is spread
across queues: sync=weights/stores, scalar=hiddens + tail stores, gpsimd=constants/rope tables.
"""

import sys

sys.path.insert(0, "/opt/trn_rl_repo")

import math
from contextlib import ExitStack

import ml_dtypes
import numpy as np

BF = ml_dtypes.bfloat16

import concourse.bass as bass
import concourse.mybir as mybir
import concourse.tile as tile
from concourse import bacc
from concourse.bass_utils import run_bass_kernel_spmd

F32 = mybir.dt.float32
BF16 = mybir.dt.bfloat16
FP16 = mybir.dt.float16
I32 = mybir.dt.int32
AF = mybir.ActivationFunctionType
OP = mybir.AluOpType

H = 2048          # hidden size
S = 2048          # sequence length
D = 128           # head dim
NH_L = 4          # q heads per rank
QW = NH_L * D     # 512 local q width
CW = QW + 2 * D   # 768 local qkv width
P = 128
KO = H // P       # 16 contraction tiles
ST = S // 512     # 4 seq tiles of 512
SM_SCALE = float(D) ** -0.5
EPS = 1e-6
ROPE_THETA = 10000.0
NEG = -1.0e9


def _build():
    nc = bacc.Bacc("TRN2", target_bir_lowering=False, debug=False, num_devices=8)

    hT = nc.dram_tensor("hT", [H, S], BF16, kind="ExternalInput").ap()
    wqkv = nc.dram_tensor("wqkv", [H, CW], BF16, kind="ExternalInput").ap()
    wo = nc.dram_tensor("wo", [QW, H], BF16, kind="ExternalInput").ap()
    cosbt = nc.dram_tensor("cosbt", [P, S], F32, kind="ExternalInput").ap()
    sinbt = nc.dram_tensor("sinbt", [P, S], F32, kind="ExternalInput").ap()
    wqn = nc.dram_tensor("wqn", [D, 1], F32, kind="ExternalInput").ap()
    wkn = nc.dram_tensor("wkn", [D, 1], F32, kind="ExternalInput").ap()
    maskneg = nc.dram_tensor("maskneg", [P, P], BF16, kind="ExternalInput").ap()
    identr = nc.dram_tensor("identr", [P, P], BF16, kind="ExternalInput").ap()
    rmat = nc.dram_tensor("rmat", [P, P], BF16, kind="ExternalInput").ap()
    out = nc.dram_tensor("out", [S, H], FP16, kind="ExternalOutput").ap()

    hT3 = hT.rearrange("(ko p) s -> p ko s", p=P)
    wqkv3 = wqkv.rearrange("(ko p) c -> p ko c", p=P)
    wo3 = wo.rearrange("(ks p) n -> p ks n", p=P)
    out3 = out.rearrange("(st p) n -> p st n", p=P)

    with ExitStack() as ctx:
        tc = ctx.enter_context(tile.TileContext(nc))
        consts = ctx.enter_context(tc.tile_pool(name="consts", bufs=1))
        qkvp = ctx.enter_context(tc.tile_pool(name="qkvp", bufs=1))
        vtp = ctx.enter_context(tc.tile_pool(name="vtp", bufs=1))
        abp = ExitStack()
        csp = abp.enter_context(tc.tile_pool(name="csp", bufs=1))

        # constants on the gpsimd (SWDGE) queue; sync queue starts on
        # weights.  cos/sin rope tables are host-computed (exact) and
        # DMA'd in per-st chunks so the big table transfers don't steal
        # HBM bandwidth from the critical first weight/hidden stream.
        wqn_sb = consts.tile([D, 1], F32)
        nc.gpsimd.dma_start(wqn_sb, wqn)
        wkn_sb = consts.tile([D, 1], F32)
        nc.gpsimd.dma_start(wkn_sb, wkn)
        rmat_sb = consts.tile([P, P], BF16)
        nc.gpsimd.dma_start(rmat_sb, rmat)
        identr_sb = consts.tile([P, P], BF16)
        nc.gpsimd.dma_start(identr_sb, identr)
        maskneg_sb = consts.tile([P, P], BF16)
        nc.gpsimd.dma_start(maskneg_sb, maskneg)
        cosb = csp.tile([P, S], F32)
        sinb = csp.tile([P, S], F32)
        nc.gpsimd.dma_start(cosb[:, 0:512], cosbt[:, 0:512])
        nc.gpsimd.dma_start(sinb[:, 0:512], sinbt[:, 0:512])
        # (later st's cos/sin chunks are issued inside the st loop, after
        # that st's hiddens, in need-order on the gpsimd queue)
        ones_sb = consts.tile([P, P], BF16)
        nc.vector.memset(ones_sb, 1.0)
        eps_sb = consts.tile([P, 1], F32)
        nc.vector.memset(eps_sb, EPS)

        q_sb = qkvp.tile([P, NH_L, S], BF16)
        k_sb = qkvp.tile([P, S], BF16)
        vT_sb = vtp.tile([P, KO, P], BF16)

        wqkv_p = abp.enter_context(tc.tile_pool(name="wqkv_p", bufs=1))
        ht_p = abp.enter_context(tc.tile_pool(name="ht_p", bufs=2))
        cpool = abp.enter_context(tc.tile_pool(name="cpool", bufs=1))

        # ---- Phase A: QKV projection + fused norm/rope/v-transpose ----
        # host layout: wqkv cols = [k(128), v(128), q(512)]
        CT_ORDER = [0, 1, 2, 3, 4, 5]  # k, v, then q heads

        def chunk_stage1(x_ch, w_sb, st, ps_c, ps_q, d2):
            """sum-of-squares via full-width ones-matmul into a broadcast
            [128,512] psum (every partition = ssq), rsqrt via Ln/Exp on
            ACT, then the rope rotation.  Emitted one QKV group late so
            the matmuls' DVE inputs are ready when PE reaches them."""
            sl = slice(512 * st, 512 * (st + 1))
            xsq = cpool.tile([P, 512], BF16, tag="ctmp", bufs=4, name="xsq")
            nc.vector.tensor_mul(xsq, x_ch, x_ch)
            sq_ps = ps_q.tile([P, 512], F32, tag="sq", name="sq_ps")
            nc.tensor.matmul(sq_ps, ones_sb, xsq, start=True, stop=True)
            ln_t = cpool.tile([P, 512], F32, tag="lnt", bufs=2, name="ln_t")
            nc.scalar.activation(ln_t, sq_ps, AF.Ln, bias=eps_sb,
                                 scale=1.0 / D)
            rb = cpool.tile([P, 512], BF16, tag="rb", bufs=3, name="rb")
            nc.scalar.activation(rb, ln_t, AF.Exp, scale=-0.5)
            nc.vector.tensor_scalar_mul(x_ch, x_ch, w_sb)
            t1m = cpool.tile([P, 512], BF16, tag="ctmp", bufs=4, name="t1m")
            nc.vector.tensor_mul(t1m, x_ch, cosb[:, sl])
            qr_ps = ps_c.tile([P, 512], F32, tag="qr", name="qr_ps")
            nc.tensor.matmul(qr_ps, rmat_sb, x_ch, start=True, stop=True)
            nc.vector.tensor_tensor(x_ch, qr_ps, sinb[:, sl], OP.mult)
            nc.vector.tensor_tensor(x_ch, x_ch, t1m, OP.add)

            def s2(x_ch=x_ch, rb=rb):
                nc.vector.tensor_tensor(x_ch, x_ch, rb, OP.mult)
            d2.append(s2)

        with nc.named_scope("qkv_proj"):
            with tc.tile_pool(name="ps_a", bufs=4, space="PSUM") as ps_a, \
                 tc.tile_pool(name="ps_c", bufs=2, space="PSUM") as ps_c, \
                 tc.tile_pool(name="ps_q", bufs=2, space="PSUM") as ps_q:
                wq_sb = wqkv_p.tile([P, KO, CW], BF16)
                d1 = []  # stage-1 closures, one group late
                d2 = []  # stage-2 closures, drained one per group
                for st in range(ST):
                    ssl = slice(512 * st, 512 * (st + 1))
                    ht_sb = ht_p.tile([P, KO, 512], BF16, tag="ht",
                                      name=f"ht_{st}")
                    if st == 0:
                        # st0's critical stream is split across the sync and
                        # scalar DMA queues so two rings pull from HBM in
                        # parallel: sync carries k/v weight columns, the
                        # tail-half hiddens, then the first two q column
                        # groups; scalar carries the head-half hiddens then
                        # the last two q column groups.
                        nc.sync.dma_start(
                            wq_sb[:, 0:2, 0 : 2 * D], wqkv3[:, 0:2, 0 : 2 * D])
                        nc.scalar.dma_start(ht_sb[:, 0:2], hT3[:, 0:2, ssl])
                        nc.sync.dma_start(
                            wq_sb[:, 2:4, 0 : 2 * D], wqkv3[:, 2:4, 0 : 2 * D])
                        nc.scalar.dma_start(ht_sb[:, 2:4], hT3[:, 2:4, ssl])
                        nc.sync.dma_start(
                            wq_sb[:, 4:8, 0 : 2 * D], wqkv3[:, 4:8, 0 : 2 * D])
                        nc.scalar.dma_start(ht_sb[:, 4:8], hT3[:, 4:8, ssl])
                        nc.sync.dma_start(
                            wq_sb[:, 8:KO, 0 : 2 * D],
                            wqkv3[:, 8:KO, 0 : 2 * D])
                        nc.sync.dma_start(ht_sb[:, 8:12], hT3[:, 8:12, ssl])
                        nc.sync.dma_start(ht_sb[:, 12:16], hT3[:, 12:16, ssl])
                        for qc in range(4):
                            qs = slice(2 * D + P * qc, 2 * D + P * (qc + 1))
                            q_eng = nc.sync if qc < 2 else nc.scalar
                            q_eng.dma_start(wq_sb[:, :, qs], wqkv3[:, :, qs])
                    else:
                        # later hiddens + their rope tables ride the gpsimd
                        # queue in need-order (consts finish by ~4us)
                        nc.gpsimd.dma_start(ht_sb[:, 0:8], hT3[:, 0:8, ssl])
                        nc.gpsimd.dma_start(ht_sb[:, 8:16], hT3[:, 8:16, ssl])
                        nc.gpsimd.dma_start(cosb[:, ssl], cosbt[:, ssl])
                        nc.gpsimd.dma_start(sinb[:, ssl], sinbt[:, ssl])
                    for ct in CT_ORDER:
                        acc = ps_a.tile([P, 512], F32, tag="qkv_ps",
                                        name=f"qkv_ps_{st}_{ct}")
                        for ko in range(KO):
                            nc.tensor.matmul(
                                acc,
                                wq_sb[:, ko, P * ct : P * (ct + 1)],
                                ht_sb[:, ko],
                                start=(ko == 0),
                                stop=(ko == KO - 1),
                            )
                        # psum->sbuf copies run on ACT (idle here), keeping
                        # DVE for the rope/rms elementwise chain
                        if ct == 0:
                            x_ch, w_sb = k_sb[:, ssl], wkn_sb
                            nc.scalar.copy(x_ch, acc)
                        elif ct == 1:
                            vch = cpool.tile([P, 512], BF16, tag="vch", bufs=2,
                                             name="vch")
                            nc.scalar.copy(vch, acc)
                        else:
                            x_ch, w_sb = q_sb[:, ct - 2, ssl], wqn_sb
                            nc.scalar.copy(x_ch, acc)

                        # run pipelined stages of earlier chunks
                        if d1:
                            d1.pop(0)()
                        if d2:
                            d2.pop(0)()

                        if ct == 1:
                            def vtrans(vch=vch, st=st):
                                for i in range(4):
                                    vt_ps = ps_c.tile([P, P], BF16, tag="qr",
                                                      name="vt_ps")
                                    nc.tensor.transpose(
                                        vt_ps, vch[:, P * i : P * (i + 1)],
                                        identr_sb)
                                    nc.vector.tensor_copy(
                                        vT_sb[:, 4 * st + i], vt_ps)
                            d1.append(vtrans)
                        else:
                            def s1(x_ch=x_ch, w_sb=w_sb, st=st):
                                chunk_stage1(x_ch, w_sb, st, ps_c, ps_q, d2)
                            d1.append(s1)
                while d1:
                    d1.pop(0)()
                while d2:
                    d2.pop(0)()

        abp.close()  # release cos/sin tables + norm temps

        # ------------- Phase E/F: attention + out projection -------------
        with tc.tile_pool(name="otp", bufs=1) as otp, \
             tc.tile_pool(name="wop", bufs=1) as wop, \
             tc.tile_pool(name="expp", bufs=12) as expp, \
             tc.tile_pool(name="dip", bufs=2) as dip, \
             tc.tile_pool(name="outp", bufs=3) as outp, \
             tc.tile_pool(name="ps_st", bufs=2, space="PSUM") as ps_st, \
             tc.tile_pool(name="ps_o", bufs=2, space="PSUM") as ps_o, \
             tc.tile_pool(name="ps_dn", bufs=2, space="PSUM") as ps_dn:
            oT_sb = otp.tile([P, NH_L, S], BF16)
            wo_sb = wop.tile([P, NH_L, H], BF16)
            for ks in range(NH_L):
                nc.sync.dma_start(wo_sb[:, ks], wo3[:, ks])

            carry = []  # deferred pv/den/out-proj work from previous heads

            def attn_tile(st, hh):
                n_sk = 4 * st + 4
                qh = q_sb[:, hh, 512 * st : 512 * (st + 1)]
                o_ps = ps_o.tile([P, 512], F32, tag="o_ps", name="o_ps")
                den_ps = ps_dn.tile([P, 512], F32, tag="dn", name="den_ps")

                def vis0(j):
                    # first visible sq column of sk-block j within this
                    # 512-wide sq tile; columns below it are fully masked
                    # and never computed/read anywhere
                    return max(0, 128 * (j - 4 * st))

                def emit_den(eps):
                    # denominator: full-width ones-matmul broadcast of the
                    # partition sum of each exp tile into every psum
                    # partition (full PE rate, unlike small-M one-hots),
                    # accumulated over sk blocks into this head's bank
                    for ep, j0 in eps:
                        for u in (0, 1):
                            j = j0 + u
                            c0 = vis0(j)
                            nc.tensor.matmul(den_ps[:, c0:512], ones_sb,
                                             ep[:, u, c0:512],
                                             start=(j == 0),
                                             stop=(j == n_sk - 1))

                def emit_pv(eps):
                    for ep, j0 in eps:
                        for u in (0, 1):
                            j = j0 + u
                            c0 = vis0(j)
                            nc.tensor.matmul(o_ps[:, c0:512], vT_sb[:, j],
                                             ep[:, u, c0:512],
                                             start=(j == 0),
                                             stop=(j == n_sk - 1))

                pend = []
                for m in range(2 * st + 2):
                    sT = ps_st.tile([P, 1024], F32, tag="sT", name="sT")
                    for u in (0, 1):
                        j = 2 * m + u
                        half = sT[:, 512 * u : 512 * (u + 1)]
                        if j >= 4 * st:
                            # scores over the visible span (start marks the
                            # whole bank), then the [128,128] staircase mask
                            # accumulated on the diagonal block only
                            c0 = vis0(j)
                            nc.tensor.matmul(
                                half[:, c0:512],
                                k_sb[:, P * j : P * (j + 1)], qh[:, c0:512],
                                start=True, stop=False)
                            nc.tensor.matmul(
                                half[:, c0 : c0 + 128], identr_sb,
                                maskneg_sb,
                                start=False, stop=True)
                        else:
                            nc.tensor.matmul(
                                half, k_sb[:, P * j : P * (j + 1)], qh,
                                start=True, stop=True)
                    ep = expp.tile([P, 2, 512], BF16, tag="ep", name="ep")
                    nc.scalar.activation(ep, sT, AF.Exp, scale=SM_SCALE)
                    # interleave the previous head's deferred den/PV (its
                    # exps are long done) with this head's scores, so PE
                    # never drains a tile's den/PV right after its exps
                    pend.append((ep, 2 * m))
                    if carry:
                        carry.pop(0)()
                    elif len(pend) > 3:
                        batch, pend = pend[:2], pend[2:]
                        emit_den(batch)
                        emit_pv(batch)

                def mk(batch):
                    def go():
                        emit_den(batch)
                        emit_pv(batch)
                    return go

                while pend:
                    batch, pend = pend[:2], pend[2:]
                    carry.append(mk(batch))

                osl = oT_sb[:, hh, 512 * st : 512 * (st + 1)]

                def fin_head(den_ps=den_ps, o_ps=o_ps, osl=osl):
                    # 1/den on DVE (18-bit custom op), normalize fused into
                    # the PV psum->sbuf copy
                    di = dip.tile([P, 512], F32, tag="di", name="di")
                    nc.vector.reciprocal_approx_fast(di, den_ps)
                    nc.vector.tensor_tensor(osl, o_ps, di, OP.mult)
                carry.append(fin_head)
                return osl

            def out_proj(st):
                # the final tile's out-proj is the serial tail: attention is
                # done, so spread its psum groups over the idle scores slots
                # and its copies over both ACT and DVE; non-tail partials
                # are stored as two-tile fp16 batches to halve DMA issues
                tail = st == ST - 1
                with nc.named_scope(f"out_proj_t{st}"):
                    for tp in range(2):
                        if not tail:
                            ob = outp.tile([P, 2, H], FP16, tag="out_sb",
                                           name="out_sb")
                        for ti in range(2):
                            t = 4 * st + 2 * tp + ti
                            if tail:
                                ob1 = outp.tile([P, H], FP16, tag="out_tl",
                                                bufs=2, name="out_tl")
                            for nt in range(4):
                                if tail and nt % 2 == 1:
                                    acc = ps_st.tile([P, 512], F32, tag="sT",
                                                     name="out_ps_b")
                                else:
                                    acc = ps_o.tile([P, 512], F32, tag="o_ps",
                                                    name="out_ps")
                                for ks in range(NH_L):
                                    nc.tensor.matmul(
                                        acc,
                                        oT_sb[:, ks, P * t : P * (t + 1)],
                                        wo_sb[:, ks, 512 * nt : 512 * (nt + 1)],
                                        start=(ks == 0),
                                        stop=(ks == NH_L - 1),
                                    )
                                if tail:
                                    dst = ob1[:, 512 * nt : 512 * (nt + 1)]
                                else:
                                    dst = ob[:, ti, 512 * nt : 512 * (nt + 1)]
                                if tail and nt % 2 == 1:
                                    nc.scalar.copy(dst, acc)
                                else:
                                    nc.vector.tensor_copy(dst, acc)
                                if tail:
                                    # chunked store on alternating queues so
                                    # the last DMAs don't all trail the
                                    # final copy
                                    q = nc.sync if nt % 2 == 0 else nc.scalar
                                    q.dma_start(
                                        out3[:, t, 512 * nt : 512 * (nt + 1)],
                                        dst)
                        if not tail:
                            t0 = 4 * st + 2 * tp
                            nc.sync.dma_start(out3[:, t0 : t0 + 2], ob)

            for st in range(ST):
                for hh in range(NH_L):
                    with nc.named_scope(f"attn_h{hh}_t{st}"):
                        attn_tile(st, hh)

                def fin_st(st=st):
                    out_proj(st)
                carry.append(fin_st)
            while carry:
                carry.pop(0)()

    nc.compile()
    _merge_act_table_loads(nc)
    return nc


def _merge_act_table_loads(nc):
    """Ln(5)/Exp+Copy(0) both live in set 6 (natural_log_exp_and_others);
    bass's per-function table choice alternates 5/0 and reloads tables at
    every Ln<->Exp transition (~1.5us each).  Retarget those loads to
    set 6 and drop the now-redundant reloads."""
    for b in nc.main_func.blocks:
        loaded = None
        keep = []
        for inst in b.instructions:
            if isinstance(inst, mybir.InstLoadActFuncSet):
                tid = inst.act_func_set_id
                if tid in (0, 5):
                    tid = 6
                if tid == loaded:
                    continue
                inst.act_func_set_id = tid
                loaded = tid
            keep.append(inst)
        b.instructions[:] = keep


_NC_CACHE = None


def _get_nc():
    global _NC_CACHE
    if _NC_CACHE is None:
        _NC_CACHE = _build()
    return _NC_CACHE


def _host_inputs(positions, hidden_states, w_qkv, w_o, q_norm_w, k_norm_w):
    """Build the 8 per-core input maps."""
    positions = np.asarray(positions, dtype=np.int32)
    hidden_states = np.asarray(hidden_states, dtype=np.float32)
    w_qkv = np.asarray(w_qkv, dtype=np.float32)
    w_o = np.asarray(w_o, dtype=np.float32)
    q_norm_w = np.asarray(q_norm_w, dtype=np.float32)
    k_norm_w = np.asarray(k_norm_w, dtype=np.float32)

    invf = 1.0 / (ROPE_THETA ** (np.arange(0, D, 2, dtype=np.float64) / D))
    p_idx = np.arange(P).reshape(P, 1)
    c_idx = np.arange(P).reshape(1, P)
    maskneg = np.where(p_idx > c_idx, np.float32(NEG), np.float32(0.0))
    maskneg = maskneg.astype(BF)
    identr = np.eye(P, dtype=BF)
    rmat = np.zeros((P, P), dtype=BF)
    for i in range(64):
        rmat[64 + i, i] = -1.0
        rmat[i, 64 + i] = 1.0
    wqn = q_norm_w.reshape(D, 1)
    wkn = k_norm_w.reshape(D, 1)

    # host-exact rope tables per batch: row r (r%64 = freq) x position
    cosbt, sinbt = [], []
    for g in range(positions.shape[0]):
        ang = np.outer(invf, positions[g].astype(np.float64))  # [64, S]
        c = np.cos(ang).astype(np.float32)
        s = np.sin(ang).astype(np.float32)
        cosbt.append(np.concatenate([c, c], axis=0))
        sinbt.append(np.concatenate([s, s], axis=0))

    in_maps = []
    for core in range(8):
        g, r = core // 4, core % 4
        wq_cols = w_qkv[:, 512 * r : 512 * (r + 1)]
        wk_col = w_qkv[:, 2048 + 128 * r : 2048 + 128 * (r + 1)]
        wv_col = w_qkv[:, 2560 + 128 * r : 2560 + 128 * (r + 1)]
        in_maps.append(
            {
                "hT": np.ascontiguousarray(hidden_states[g].T).astype(BF),
                "wqkv": np.ascontiguousarray(
                    np.concatenate([wk_col, wv_col, wq_cols], axis=1)
                ).astype(BF),
                "wo": np.ascontiguousarray(
                    w_o[512 * r : 512 * (r + 1), :]
                ).astype(BF),
                "cosbt": cosbt[g],
                "sinbt": sinbt[g],
                "wqn": wqn,
                "wkn": wkn,
                "maskneg": maskneg,
                "identr": identr,
                "rmat": rmat,
            }
        )
    return in_maps


def run(trace=False, **inputs):
    nc = _get_nc()
    in_maps = _host_inputs(**inputs)
    res = run_bass_kernel_spmd(nc, in_maps, core_ids=list(range(8)), trace=trace)
    B = inputs["hidden_states"].shape[0]
    out = np.zeros((B, S, H), dtype=np.float64)
    for core in range(8):
        g = core // 4
        out[g] += res.results[core]["out"].astype(np.float64)
    return out.astype(np.float32), res


def kernel(**inputs):
    out, _ = run(trace=False, **inputs)
    return out
